# revision 9
# baseline (speedup 1.0000x reference)
"""Viterbi decode (CRF layer) on Trainium2 — Bass kernel.

Problem: feats [1024, 512, 128] f32, transitions [128, 128],
start/stop_transitions [128] -> best tag sequence [1024, 512] int32.

Strategy: pure batch data-parallelism across 8 NeuronCores. Each core takes
128 batch rows (= 128 SBUF partitions) and runs the sequential max-plus
forward scan on-chip:

    sc[b, i, j] = v[b, i] + trans[i, j]          (fp32, one rounding)
    mx[b, j]    = max_i sc[b, i, j]
    v'[b, j]    = mx[b, j] + feats[b, t, j]      (fp32, one rounding)

The per-step state vectors v stream to DRAM; the backtrace recomputes the
argmax only along the traced path (B*S tiny argmaxes) on host during the
unshard step, with identical fp32 arithmetic and first-index tie-breaking,
so the final int32 tags match the reference bit-exactly.

variant="v0" keeps the full device-side backpointer computation (slower,
fully self-contained backpointers) as a fallback.
"""

import numpy as np

B, S, T = 1024, 512, 128
NCORES = 8
BL = B // NCORES  # 128 batch rows per core == SBUF partition count


def build_viterbi_nc(trans_np, S_=S, T_=T, BL_=BL, variant="v1"):
    """Build the per-core Bass program (same NEFF for all cores).

    NOTE: start_transitions must already be folded into feats[:, 0, :] by the
    caller (bit-exact: same single fp32 add the reference performs).

    walrus/core_v3 allows only ONE attached sync-wait per compute
    instruction; the initial state goes through a DVE tensor_copy so every
    instruction waits on at most one foreign semaphore.
    """
    import concourse.bacc as bacc
    import concourse.mybir as mybir
    import concourse.tile as tile

    f32 = mybir.dt.float32
    add = mybir.AluOpType.add
    mx_op = mybir.AluOpType.max
    eq_op = mybir.AluOpType.is_equal
    mul_op = mybir.AluOpType.mult
    X = mybir.AxisListType.X

    nc = bacc.Bacc("TRN2", target_bir_lowering=False, debug=False)
    feats = nc.declare_dram_parameter("feats", [BL_, S_, T_], f32, isOutput=False)
    if variant == "v0":
        bp = nc.declare_dram_parameter("bp", [S_ - 1, BL_, T_], f32, isOutput=True)
    else:
        vs_out = nc.declare_dram_parameter("vs", [S_ - 1, BL_, T_], f32, isOutput=True)
    v_final = nc.declare_dram_parameter("v_final", [BL_, T_], f32, isOutput=True)

    if variant == "v2":
        # table stored [j, i] (transposed) so the score buffer is written and
        # reduced fully contiguously in [b, j, i] order
        tbl = np.ascontiguousarray(trans_np.T.reshape(1, T_ * T_), dtype=np.float32)
    else:
        tbl = np.ascontiguousarray(trans_np.reshape(1, T_ * T_), dtype=np.float32)
    tbc_d = nc.inline_tensor(tbl, "tbc")
    iota_d = nc.inline_tensor(
        np.arange(T_ - 1, -1, -1, dtype=np.float32).reshape(1, T_), "iotad"
    )

    with tile.TileContext(nc) as tc:
        with (
            tc.tile_pool(name="const", bufs=1) as cpool,
            tc.tile_pool(name="feat", bufs=8) as fpool,
            tc.tile_pool(name="vst", bufs=4) as vpool,
            tc.tile_pool(name="sc", bufs=1 if variant == "v0" else 2) as scpool,
            tc.tile_pool(name="mx", bufs=2) as mxpool,
            tc.tile_pool(name="bpp", bufs=4) as bppool,
        ):
            tbc = cpool.tile([BL_, T_ * T_], f32, tag="tbc")
            nc.gpsimd.dma_start(tbc[:, :], tbc_d[:, :].partition_broadcast(BL_))
            iotab = cpool.tile([BL_, T_], f32, tag="iotab")
            nc.gpsimd.dma_start(iotab[:, :], iota_d[:, :].partition_broadcast(BL_))

            f0 = fpool.tile([BL_, T_], f32, tag="feat")
            nc.gpsimd.dma_start(f0[:, :], feats[:, 0, :])
            v = vpool.tile([BL_, T_], f32, tag="v")
            nc.vector.tensor_copy(v[:, :], f0[:, :])

            tb3 = tbc[:, :].rearrange("p (i j) -> p i j", i=T_)
            io3 = iotab[:, :].unsqueeze(-1).broadcast_to([BL_, T_, T_])
            # v2: table is [j, i]-major; split the add by j between DVE and
            # Pool (Pool ~2x slower -> give it the smaller range)
            import os as _os
            JSPLIT = int(_os.environ.get("VT_JSPLIT", T_))
            DSPLIT = int(_os.environ.get("VT_DSPLIT", T_ // 2))

            for t in range(1, S_):
                ft = fpool.tile([BL_, T_], f32, tag="feat")
                nc.gpsimd.dma_start(ft[:, :], feats[:, t, :])

                sc = scpool.tile([BL_, T_ * T_], f32, tag="sc")
                sc3 = sc[:, :].rearrange("p (i j) -> p i j", i=T_)
                scT = sc[:, :].rearrange("p (i j) -> p j i", i=T_)
                mxt = mxpool.tile([BL_, T_], f32, tag="mx")

                if variant == "v2":
                    # sc[b, j, i] = v[b, i] + tT[j, i]; contiguous writes
                    scJ = sc[:, :].rearrange("p (j i) -> p j i", j=T_)
                    tbJ = tbc[:, :].rearrange("p (j i) -> p j i", j=T_)
                    nA = JSPLIT * T_
                    v3a = v[:, :].unsqueeze(1).broadcast_to([BL_, JSPLIT, T_])
                    scA = sc[:, 0:nA].rearrange("p (j i) -> p j i", j=JSPLIT)
                    tbA = tbc[:, 0:nA].rearrange("p (j i) -> p j i", j=JSPLIT)
                    nc.vector.tensor_tensor(scA, v3a, tbA, add)
                    if JSPLIT < T_:
                        v3b = v[:, :].unsqueeze(1).broadcast_to(
                            [BL_, T_ - JSPLIT, T_]
                        )
                        scB = sc[:, nA : T_ * T_].rearrange(
                            "p (j i) -> p j i", j=T_ - JSPLIT
                        )
                        tbB = tbc[:, nA : T_ * T_].rearrange(
                            "p (j i) -> p j i", j=T_ - JSPLIT
                        )
                        nc.gpsimd.tensor_tensor(scB, v3b, tbB, add)
                    nc.vector.tensor_reduce(mxt[:, :], scJ, axis=X, op=mx_op)
                elif variant == "v3":
                    # sc[b,i,j] = t[i,j] + v[b,i]: DVE does rows [0, DSPLIT)
                    # in one tensor_tensor; ACT does rows [DSPLIT, T) as
                    # per-row activation-adds (bias = per-partition scalar)
                    nD = DSPLIT * T_
                    v3a = v[:, 0:DSPLIT].unsqueeze(-1).broadcast_to(
                        [BL_, DSPLIT, T_]
                    )
                    scA = sc[:, 0:nD].rearrange("p (i j) -> p i j", i=DSPLIT)
                    tbA = tbc[:, 0:nD].rearrange("p (i j) -> p i j", i=DSPLIT)
                    nc.vector.tensor_tensor(scA, v3a, tbA, add)
                    for i in range(DSPLIT, T_):
                        nc.scalar.add(
                            sc[:, i * T_ : (i + 1) * T_],
                            tbc[:, i * T_ : (i + 1) * T_],
                            v[:, i : i + 1],
                        )
                    nc.vector.tensor_reduce(mxt[:, :], scT, axis=X, op=mx_op)
                else:
                    v3 = v[:, :].unsqueeze(-1).broadcast_to([BL_, T_, T_])
                    nc.vector.tensor_tensor(sc3, v3, tb3, add)
                    nc.vector.tensor_reduce(mxt[:, :], scT, axis=X, op=mx_op)

                vn = vpool.tile([BL_, T_], f32, tag="v")
                nc.vector.tensor_tensor(vn[:, :], mxt[:, :], ft[:, :], add)

                if variant == "v0":
                    # backpointers on device: sc <- (sc==mx)*(T-1-i); bp=max_i
                    mx3 = mxt[:, :].unsqueeze(1).broadcast_to([BL_, T_, T_])
                    nc.vector.tensor_tensor(sc3, sc3, mx3, eq_op)
                    nc.vector.tensor_tensor(sc3, sc3, io3, mul_op)
                    bpt = bppool.tile([BL_, T_], f32, tag="bp")
                    nc.vector.tensor_reduce(bpt[:, :], scT, axis=X, op=mx_op)
                    nc.gpsimd.dma_start(bp[t - 1, :, :], bpt[:, :])
                else:
                    nc.gpsimd.dma_start(vs_out[t - 1, :, :], vn[:, :])

                v = vn

            nc.gpsimd.dma_start(v_final[:, :], v[:, :])
    nc.finalize()
    return nc


def build_viterbi_f16_nc(trans_np, S_=S, T_=T, BL_=BL, kblk=8):
    """fp16 forward-scan kernel: per step, the [T,T] score add and the
    max-tree run in fp16 on DVE (4x perf mode); the state update
    vn = max + feat stays fp32, with per-step recentring (subtract the
    per-row max) so fp16 magnitudes stay ~|8|. The recentred fp32 state
    trajectory streams to DRAM b-major; the host backtraces from it in fp32.

    Numerics validated against reference in numpy sim: ~1e-4 tag mismatch
    rate (rel err ~7e-3, gate is 2e-2).
    """
    import concourse.bacc as bacc
    import concourse.mybir as mybir
    import concourse.tile as tile

    f32 = mybir.dt.float32
    f16 = mybir.dt.float16
    add = mybir.AluOpType.add
    mx_op = mybir.AluOpType.max
    mul_op = mybir.AluOpType.mult
    sub_op = mybir.AluOpType.subtract
    X = mybir.AxisListType.X

    nc = bacc.Bacc("TRN2", target_bir_lowering=False, debug=False)
    feats = nc.declare_dram_parameter("feats", [BL_, S_, T_], f32, isOutput=False)
    vs_out = nc.declare_dram_parameter("vs", [BL_, S_ - 1, T_], f32, isOutput=True)

    # table stored [j, i] (transposed) so score writes and the i-tree are
    # contiguous per j
    tbl16 = np.ascontiguousarray(trans_np.T.reshape(1, T_ * T_)).astype(np.float16)
    tbc_d = nc.inline_tensor(tbl16, "tbc16")

    nblk = (S_ + kblk - 1) // kblk  # feat blocks cover s in [0, S)

    with tile.TileContext(nc) as tc:
        with (
            tc.tile_pool(name="const", bufs=1) as cpool,
            tc.tile_pool(name="feat", bufs=2) as fpool,
            tc.tile_pool(name="vsb", bufs=2) as vspool,
            tc.tile_pool(name="sc", bufs=1) as scpool,
            tc.tile_pool(name="small", bufs=2) as smpool,
        ):
            tbc = cpool.tile([BL_, T_ * T_], f16, tag="tbc")
            nc.gpsimd.dma_start(tbc[:, :], tbc_d[:, :].partition_broadcast(BL_))
            t3 = tbc[:, :].rearrange("p (j i) -> p j i", j=T_)

            s16 = scpool.tile([BL_, T_ * T_], f16, tag="s16")
            s3 = s16[:, :].rearrange("p (j i) -> p j i", j=T_)

            # feat block 0 (s = 0..kblk-1)
            fb = fpool.tile([BL_, kblk * T_], f32, tag="fb")
            nc.gpsimd.dma_start(
                fb[:, :].rearrange("p (k t) -> p k t", k=kblk), feats[:, 0:kblk, :]
            )

            # initial state from f0 (host already folded start_transitions)
            f0 = fb[:, 0:T_]
            shift = smpool.tile([BL_, 1], f32, tag="shift")
            nc.vector.tensor_reduce(shift[:, :], f0, axis=X, op=mx_op)
            v16 = smpool.tile([BL_, T_], f16, tag="v16")
            nc.vector.tensor_scalar(v16[:, :], f0, shift[:, :], None, sub_op)

            vsb = vspool.tile([BL_, kblk * T_], f32, tag="vsb")

            for t in range(1, S_):
                kf, rf = divmod(t, kblk)
                if rf == 0:  # need next feat block (covers s = t..t+kblk-1)
                    fb = fpool.tile([BL_, kblk * T_], f32, tag="fb")
                    hi = min(kblk, S_ - kf * kblk)
                    nc.gpsimd.dma_start(
                        fb[:, 0 : hi * T_].rearrange("p (k t) -> p k t", k=hi),
                        feats[:, kf * kblk : kf * kblk + hi, :],
                    )
                ft = fb[:, rf * T_ : (rf + 1) * T_]

                # s16[b,j,i] = fp16(v16[b,i] + t16[j,i])   (4x DVE mode)
                v3 = v16[:, :].unsqueeze(1).broadcast_to([BL_, T_, T_])
                nc.vector.scalar_tensor_tensor(s3, v3, 1.0, t3, mul_op, add)

                # in-place max tree over i: 128 -> 1
                w = T_ // 2
                while w >= 1:
                    a = s3[:, :, 0:w]
                    b = s3[:, :, w : 2 * w]
                    if w > 1:
                        nc.vector.scalar_tensor_tensor(a, a, 1.0, b, mul_op, mx_op)
                    else:
                        mx16 = smpool.tile([BL_, T_], f16, tag="mx16")
                        m3 = mx16[:, :].rearrange("p (j i) -> p j i", j=T_, i=1)
                        nc.vector.scalar_tensor_tensor(m3, a, 1.0, b, mul_op, mx_op)
                    w //= 2

                # vn32 = fp32(mx16) + ft   -> written into the vs block slot
                r = (t - 1) % kblk
                if r == 0:
                    vsb = vspool.tile([BL_, kblk * T_], f32, tag="vsb")
                vslot = vsb[:, r * T_ : (r + 1) * T_]
                nc.vector.scalar_tensor_tensor(vslot, mx16[:, :], 1.0, ft, mul_op, add)

                # recentre: shift = max_j vn; v16 = fp16(vn - shift)
                shift = smpool.tile([BL_, 1], f32, tag="shift")
                nc.vector.tensor_reduce(shift[:, :], vslot, axis=X, op=mx_op)
                v16 = smpool.tile([BL_, T_], f16, tag="v16")
                nc.vector.tensor_scalar(v16[:, :], vslot, shift[:, :], None, sub_op)

                if r == kblk - 1 or t == S_ - 1:  # flush vs block
                    lo = (t - 1) - r  # first vs row in this block
                    n = r + 1
                    nc.gpsimd.dma_start(
                        vs_out[:, lo : lo + n, :],
                        vsb[:, 0 : n * T_].rearrange("p (k t) -> p k t", k=n),
                    )
    nc.finalize()
    return nc


def build_viterbi_v4_nc(trans_np, S_=S, T_=T, BL_=BL, kblk=8, a_rows=58,
                        apad=64):
    """3-engine fp32 kernel. Exact arithmetic (same single-rounding adds as
    the reference), so tags match bit-exactly.

    Score add s[b,i,j] = t[i,j] + v[b,i] split by i-rows:
      - Act: rows [0, a)   -> s_A buffer, [i, j] layout (row-contig), one
        activation-add per row (bias = v[:, i]). Rows [a, apad) are -1e38
        pad written once so DVE can run a fixed power-of-2 max tree.
      - Pool: rows [a, T)  -> s_P buffer in compact TRANSPOSED [j, k] layout
        (one tensor_tensor add; Pool has no max op, DVE reduces contiguous).
    DVE: in-place contiguous max tree over s_A rows (apad -> 1), one
    contiguous tensor_reduce over s_P, combine, + feat -> vs block slot.
    State trajectory streams b-major; host does the exact fp32 backtrace.
    """
    import concourse.bacc as bacc
    import concourse.mybir as mybir
    import concourse.tile as tile

    f32 = mybir.dt.float32
    add = mybir.AluOpType.add
    mx_op = mybir.AluOpType.max
    mul_op = mybir.AluOpType.mult
    X = mybir.AxisListType.X

    p_rows = T_ - a_rows
    assert a_rows <= apad and (apad & (apad - 1)) == 0

    nc = bacc.Bacc("TRN2", target_bir_lowering=False, debug=False)
    feats = nc.declare_dram_parameter("feats", [BL_, S_, T_], f32, isOutput=False)
    vs_out = nc.declare_dram_parameter("vs", [BL_, S_ - 1, T_], f32, isOutput=True)

    # Act table: rows [0, a) of trans, [i, j] layout
    tblA = np.ascontiguousarray(trans_np[0:a_rows, :].reshape(1, a_rows * T_),
                                dtype=np.float32)
    tblA_d = nc.inline_tensor(tblA, "tblA")
    # Pool table: rows [a, T) transposed-compact: tP[j, k] = trans[a+k, j]
    tblP = np.ascontiguousarray(trans_np[a_rows:, :].T.reshape(1, T_ * p_rows),
                                dtype=np.float32)
    tblP_d = nc.inline_tensor(tblP, "tblP")

    with tile.TileContext(nc) as tc:
        with (
            tc.tile_pool(name="const", bufs=1) as cpool,
            tc.tile_pool(name="feat", bufs=2) as fpool,
            tc.tile_pool(name="vsb", bufs=2) as vspool,
            tc.tile_pool(name="sc", bufs=1) as scpool,
            tc.tile_pool(name="small", bufs=2) as smpool,
        ):
            tbA = cpool.tile([BL_, a_rows * T_], f32, tag="tbA")
            nc.gpsimd.dma_start(tbA[:, :], tblA_d[:, :].partition_broadcast(BL_))
            tbP = cpool.tile([BL_, T_ * p_rows], f32, tag="tbP")
            nc.gpsimd.dma_start(tbP[:, :], tblP_d[:, :].partition_broadcast(BL_))
            tbP3 = tbP[:, :].rearrange("p (j k) -> p j k", j=T_)

            sA = scpool.tile([BL_, apad * T_], f32, tag="sA")
            sA3 = sA[:, :].rearrange("p (i j) -> p i j", i=apad)
            if a_rows < apad:  # one-time -inf pad rows for the fixed tree
                nc.vector.memset(sA[:, a_rows * T_ :], -1.0e38)
            sP = scpool.tile([BL_, T_ * p_rows], f32, tag="sP")
            sP3 = sP[:, :].rearrange("p (j k) -> p j k", j=T_)

            fb = fpool.tile([BL_, kblk * T_], f32, tag="fb")
            nc.gpsimd.dma_start(
                fb[:, :].rearrange("p (k t) -> p k t", k=kblk), feats[:, 0:kblk, :]
            )
            v = fb[:, 0:T_]  # v_0 = feats[:,0] (start folded by host)

            vsb = vspool.tile([BL_, kblk * T_], f32, tag="vsb")

            for t in range(1, S_):
                kf, rf = divmod(t, kblk)
                if rf == 0:
                    fb = fpool.tile([BL_, kblk * T_], f32, tag="fb")
                    hi = min(kblk, S_ - kf * kblk)
                    nc.gpsimd.dma_start(
                        fb[:, 0 : hi * T_].rearrange("p (k t) -> p k t", k=hi),
                        feats[:, kf * kblk : kf * kblk + hi, :],
                    )
                ft = fb[:, rf * T_ : (rf + 1) * T_]

                # --- score adds ---
                for i in range(a_rows):
                    nc.scalar.add(
                        sA[:, i * T_ : (i + 1) * T_],
                        tbA[:, i * T_ : (i + 1) * T_],
                        v[:, i : i + 1],
                    )
                vP = v[:, a_rows:T_].unsqueeze(1).broadcast_to([BL_, T_, p_rows])
                nc.gpsimd.tensor_tensor(sP3, tbP3, vP, add)

                # --- max over i ---
                # in-place contiguous tree over sA rows: apad -> 1
                w = apad // 2
                mxA = smpool.tile([BL_, T_], f32, tag="mxA")
                while w >= 1:
                    i0 = sA3[:, 0:w, :]
                    i1 = sA3[:, w : 2 * w, :]
                    out = i0 if w > 1 else mxA[:, :].rearrange(
                        "p (i j) -> p i j", i=1
                    )
                    nc.vector.scalar_tensor_tensor(out, i0, 1.0, i1, mul_op, mx_op)
                    w //= 2
                mxP = smpool.tile([BL_, T_], f32, tag="mxP")
                nc.vector.tensor_reduce(mxP[:, :], sP3, axis=X, op=mx_op)

                # --- combine + feat -> vs slot (the new v) ---
                r = (t - 1) % kblk
                if r == 0:
                    vsb = vspool.tile([BL_, kblk * T_], f32, tag="vsb")
                vslot = vsb[:, r * T_ : (r + 1) * T_]
                nc.vector.scalar_tensor_tensor(
                    mxA[:, :], mxA[:, :], 1.0, mxP[:, :], mul_op, mx_op
                )
                nc.vector.scalar_tensor_tensor(
                    vslot, mxA[:, :], 1.0, ft, mul_op, add
                )
                v = vslot

                if r == kblk - 1 or t == S_ - 1:
                    lo = (t - 1) - r
                    n = r + 1
                    nc.gpsimd.dma_start(
                        vs_out[:, lo : lo + n, :],
                        vsb[:, 0 : n * T_].rearrange("p (k t) -> p k t", k=n),
                    )
    nc.finalize()
    return nc


def build_viterbi_v5_nc(trans_np, S_=S, T_=T, BL_=BL, kblk=8, a_rows=32,
                        d_rows=16):
    """Pipelined 3-engine fp32 kernel (exact arithmetic).

    Row split of the score add s[b,i,j] = t[i,j] + v[b,i]:
      - Act rows [0, a): per-row activation adds into sA ([i,j] layout).
      - DVE rows [a, a+d) and Pool rows [a+d, T): both write one shared
        compact transposed buffer sDP[b, j, k] (k = i - a), so ONE
        contiguous tensor_reduce covers both regions.
    DVE owns all maxes: in-place tree over sA (a must be a power of two),
    contiguous reduce over sDP in two j-halves, combine + feat per half.
    vn half 0 (j < T/2) is emitted first so Act's next-step rows (i < a <=
    T/2) and DVE's own adds can start while the second half is still being
    reduced — that cross-step overlap is what keeps Act/Pool busy during
    DVE's reduce phase.
    """
    import concourse.bacc as bacc
    import concourse.mybir as mybir
    import concourse.tile as tile

    f32 = mybir.dt.float32
    add = mybir.AluOpType.add
    mx_op = mybir.AluOpType.max
    mul_op = mybir.AluOpType.mult
    X = mybir.AxisListType.X

    p_rows = T_ - a_rows - d_rows
    dp = d_rows + p_rows
    H = T_ // 2
    assert (a_rows & (a_rows - 1)) == 0 and a_rows <= H

    nc = bacc.Bacc("TRN2", target_bir_lowering=False, debug=False)
    feats = nc.declare_dram_parameter("feats", [BL_, S_, T_], f32, isOutput=False)
    vs_out = nc.declare_dram_parameter("vs", [BL_, S_ - 1, T_], f32, isOutput=True)

    tblA = np.ascontiguousarray(trans_np[0:a_rows, :].reshape(1, a_rows * T_),
                                dtype=np.float32)
    tblA_d = nc.inline_tensor(tblA, "tblA")
    # shared compact transposed table: tDP[j, k] = trans[a + k, j]
    tblDP = np.ascontiguousarray(trans_np[a_rows:, :].T.reshape(1, T_ * dp),
                                 dtype=np.float32)
    tblDP_d = nc.inline_tensor(tblDP, "tblDP")

    with tile.TileContext(nc) as tc:
        with (
            tc.tile_pool(name="const", bufs=1) as cpool,
            tc.tile_pool(name="feat", bufs=2) as fpool,
            tc.tile_pool(name="vsb", bufs=2) as vspool,
            tc.tile_pool(name="sc", bufs=1) as scpool,
            tc.tile_pool(name="small", bufs=2) as smpool,
        ):
            tbA = cpool.tile([BL_, a_rows * T_], f32, tag="tbA")
            nc.gpsimd.dma_start(tbA[:, :], tblA_d[:, :].partition_broadcast(BL_))
            tbDP = cpool.tile([BL_, T_ * dp], f32, tag="tbDP")
            nc.gpsimd.dma_start(tbDP[:, :], tblDP_d[:, :].partition_broadcast(BL_))
            tbDP3 = tbDP[:, :].rearrange("p (j k) -> p j k", j=T_)

            sA = scpool.tile([BL_, a_rows * T_], f32, tag="sA")
            sA3 = sA[:, :].rearrange("p (i j) -> p i j", i=a_rows)
            sDP = scpool.tile([BL_, T_ * dp], f32, tag="sDP")
            sDP3 = sDP[:, :].rearrange("p (j k) -> p j k", j=T_)

            fb = fpool.tile([BL_, kblk * T_], f32, tag="fb")
            nc.gpsimd.dma_start(
                fb[:, :].rearrange("p (k t) -> p k t", k=kblk), feats[:, 0:kblk, :]
            )
            v = fb[:, 0:T_]  # v_0 = feats[:,0] (start folded by host)

            vsb = vspool.tile([BL_, kblk * T_], f32, tag="vsb")

            D0, D1 = a_rows, a_rows + d_rows
            for t in range(1, S_):
                kf, rf = divmod(t, kblk)
                if rf == 0:
                    fb = fpool.tile([BL_, kblk * T_], f32, tag="fb")
                    hi = min(kblk, S_ - kf * kblk)
                    nc.gpsimd.dma_start(
                        fb[:, 0 : hi * T_].rearrange("p (k t) -> p k t", k=hi),
                        feats[:, kf * kblk : kf * kblk + hi, :],
                    )
                ft = fb[:, rf * T_ : (rf + 1) * T_]

                # --- score adds (Act needs only vn half 0 of step t-1) ---
                for i in range(a_rows):
                    nc.scalar.add(
                        sA[:, i * T_ : (i + 1) * T_],
                        tbA[:, i * T_ : (i + 1) * T_],
                        v[:, i : i + 1],
                    )
                if d_rows:
                    vD = v[:, D0:D1].unsqueeze(1).broadcast_to([BL_, T_, d_rows])
                    nc.vector.scalar_tensor_tensor(
                        sDP3[:, :, 0:d_rows], tbDP3[:, :, 0:d_rows], 1.0, vD,
                        mul_op, add,
                    )
                vP = v[:, D1:T_].unsqueeze(1).broadcast_to([BL_, T_, p_rows])
                nc.gpsimd.tensor_tensor(
                    sDP3[:, :, d_rows:dp], tbDP3[:, :, d_rows:dp], vP, add
                )

                # --- maxes on DVE ---
                mxA = smpool.tile([BL_, T_], f32, tag="mxA")
                w = a_rows // 2
                while w >= 1:
                    i0 = sA3[:, 0:w, :]
                    i1 = sA3[:, w : 2 * w, :]
                    out = i0 if w > 1 else mxA[:, :].rearrange(
                        "p (i j) -> p i j", i=1
                    )
                    nc.vector.scalar_tensor_tensor(out, i0, 1.0, i1, mul_op, mx_op)
                    w //= 2

                r = (t - 1) % kblk
                if r == 0:
                    vsb = vspool.tile([BL_, kblk * T_], f32, tag="vsb")
                vslot = vsb[:, r * T_ : (r + 1) * T_]
                mxP = smpool.tile([BL_, T_], f32, tag="mxP")
                for h0, h1 in ((0, H), (H, T_)):
                    nc.vector.tensor_reduce(
                        mxP[:, h0:h1], sDP3[:, h0:h1, :], axis=X, op=mx_op
                    )
                    nc.vector.scalar_tensor_tensor(
                        mxA[:, h0:h1], mxA[:, h0:h1], 1.0, mxP[:, h0:h1],
                        mul_op, mx_op,
                    )
                    nc.vector.scalar_tensor_tensor(
                        vslot[:, h0:h1], mxA[:, h0:h1], 1.0, ft[:, h0:h1],
                        mul_op, add,
                    )
                v = vslot

                if r == kblk - 1 or t == S_ - 1:
                    lo = (t - 1) - r
                    n = r + 1
                    nc.gpsimd.dma_start(
                        vs_out[:, lo : lo + n, :],
                        vsb[:, 0 : n * T_].rearrange("p (k t) -> p k t", k=n),
                    )
    nc.finalize()
    return nc


def _install_ntff_hook_shim():
    """The agent image's `antenv` lacks `axon_hooks`, so trn_boot degrades
    silently and bass_utils' trace path crashes on import. Provide the same
    ctypes-based NTFF hook trn_boot would have registered."""
    import sys
    import types

    if "antenv.axon_hooks" in sys.modules:
        return
    try:
        import antenv.axon_hooks  # noqa: F401
        return
    except ImportError:
        pass
    try:
        from trn_agent_boot.trn_boot import _ntff_profile_via_ctypes

        hook = _ntff_profile_via_ctypes("/opt/axon/libaxon_pjrt.so")
    except Exception:
        hook = None
    m = types.ModuleType("antenv.axon_hooks")
    m._hook = hook
    m.get_axon_ntff_profile_hook = lambda: m._hook
    def _set(h):
        m._hook = h
    m.set_axon_ntff_profile_hook = _set
    sys.modules["antenv.axon_hooks"] = m


def _run(nc, in_maps, **kwargs):
    kwargs.setdefault("trace", True)
    if kwargs.get("trace"):
        _install_ntff_hook_shim()
    from concourse.bass_utils import run_bass_kernel_spmd

    return run_bass_kernel_spmd(nc, in_maps, core_ids=list(range(len(in_maps))), **kwargs)


def _backtrace_from_vs(vs, v0, trans, stop):
    """Exact backtrace from per-step state vectors.

    vs: [B, S-1, T] fp32 (v at t=1..S-1), v0: [B, T] (v at t=0).
    Recomputes argmax_i(v[t-1,:,i] + trans[i, j_t]) along the traced path
    only — identical fp32 arithmetic + first-index ties as the reference.
    """
    B_, Sm1, T_ = vs.shape
    S_ = Sm1 + 1
    last = np.argmax(vs[:, -1, :] + stop[None, :], axis=1).astype(np.int32)
    tags = np.empty((B_, S_), dtype=np.int32)
    tags[:, -1] = last
    cur = last
    transT = np.ascontiguousarray(trans.T)  # [j, i]
    for t in range(S_ - 1, 0, -1):
        vprev = vs[:, t - 2, :] if t >= 2 else v0
        col = vprev + transT[cur]  # [B, T] fp32: v[b,t-1,i] + trans[i, j_t]
        cur = np.argmax(col, axis=1).astype(np.int32)
        tags[:, t - 1] = cur
    return tags


def kernel(feats, transitions, start_transitions, stop_transitions, _trace=False,
           _variant="v4"):
    feats = np.asarray(feats, dtype=np.float32).copy()
    trans = np.ascontiguousarray(np.asarray(transitions, dtype=np.float32))
    start = np.ascontiguousarray(np.asarray(start_transitions, dtype=np.float32))
    stop = np.ascontiguousarray(np.asarray(stop_transitions, dtype=np.float32))
    assert feats.shape == (B, S, T)

    feats[:, 0, :] += start  # fold start_transitions (bit-exact vs reference)

    if _variant == "v5":
        import os as _os
        nc = build_viterbi_v5_nc(
            trans,
            a_rows=int(_os.environ.get("VT_AROWS", "32")),
            d_rows=int(_os.environ.get("VT_DROWS", "16")),
            kblk=int(_os.environ.get("VT_KBLK", "8")),
        )
    elif _variant == "v4":
        import os as _os
        nc = build_viterbi_v4_nc(
            trans,
            a_rows=int(_os.environ.get("VT_AROWS", "58")),
            kblk=int(_os.environ.get("VT_KBLK", "8")),
        )
    elif _variant == "f16":
        nc = build_viterbi_f16_nc(trans)
    else:
        nc = build_viterbi_nc(trans, variant=_variant)
    in_maps = [{"feats": feats[c * BL : (c + 1) * BL]} for c in range(NCORES)]
    res = _run(nc, in_maps)

    if _variant in ("f16", "v4", "v5"):
        vs = np.concatenate([r["vs"] for r in res.results], axis=0)  # [B, S-1, T]
        v0 = feats[:, 0, :]  # start already folded
        tags = _backtrace_from_vs(vs, v0, trans, stop)
    elif _variant == "v0":
        bp_f = np.concatenate(
            [np.transpose(r["bp"], (1, 0, 2)) for r in res.results], axis=0
        )
        v_fin = np.concatenate([r["v_final"] for r in res.results], axis=0)
        idx = (T - 1) - bp_f.astype(np.int32)
        last = np.argmax(v_fin + stop[None, :], axis=1).astype(np.int32)
        tags = np.empty((B, S), dtype=np.int32)
        tags[:, S - 1] = last
        cur = last
        ar = np.arange(B)
        for t in range(S - 2, -1, -1):
            cur = idx[ar, t, cur]
            tags[:, t] = cur
    else:
        vs = np.concatenate(
            [np.transpose(r["vs"], (1, 0, 2)) for r in res.results], axis=0
        )  # [B, S-1, T]
        v0 = feats[:, 0, :]  # start already folded
        tags = _backtrace_from_vs(vs, v0, trans, stop)

    if _trace:
        return tags, res
    return tags



# revision 10
# speedup vs baseline: 1.0736x; 1.0736x over previous
"""Viterbi decode (CRF layer) on Trainium2 — Bass kernel.

Problem: feats [1024, 512, 128] f32, transitions [128, 128],
start/stop_transitions [128] -> best tag sequence [1024, 512] int32.

Strategy: pure batch data-parallelism across 8 NeuronCores. Each core takes
128 batch rows (= 128 SBUF partitions) and runs the sequential max-plus
forward scan on-chip:

    sc[b, i, j] = v[b, i] + trans[i, j]          (fp32, one rounding)
    mx[b, j]    = max_i sc[b, i, j]
    v'[b, j]    = mx[b, j] + feats[b, t, j]      (fp32, one rounding)

The per-step state vectors v stream to DRAM; the backtrace recomputes the
argmax only along the traced path (B*S tiny argmaxes) on host during the
unshard step, with identical fp32 arithmetic and first-index tie-breaking,
so the final int32 tags match the reference bit-exactly.

variant="v0" keeps the full device-side backpointer computation (slower,
fully self-contained backpointers) as a fallback.
"""

import numpy as np

B, S, T = 1024, 512, 128
NCORES = 8
BL = B // NCORES  # 128 batch rows per core == SBUF partition count


def build_viterbi_nc(trans_np, S_=S, T_=T, BL_=BL, variant="v1"):
    """Build the per-core Bass program (same NEFF for all cores).

    NOTE: start_transitions must already be folded into feats[:, 0, :] by the
    caller (bit-exact: same single fp32 add the reference performs).

    walrus/core_v3 allows only ONE attached sync-wait per compute
    instruction; the initial state goes through a DVE tensor_copy so every
    instruction waits on at most one foreign semaphore.
    """
    import concourse.bacc as bacc
    import concourse.mybir as mybir
    import concourse.tile as tile

    f32 = mybir.dt.float32
    add = mybir.AluOpType.add
    mx_op = mybir.AluOpType.max
    eq_op = mybir.AluOpType.is_equal
    mul_op = mybir.AluOpType.mult
    X = mybir.AxisListType.X

    nc = bacc.Bacc("TRN2", target_bir_lowering=False, debug=False)
    feats = nc.declare_dram_parameter("feats", [BL_, S_, T_], f32, isOutput=False)
    if variant == "v0":
        bp = nc.declare_dram_parameter("bp", [S_ - 1, BL_, T_], f32, isOutput=True)
    else:
        vs_out = nc.declare_dram_parameter("vs", [S_ - 1, BL_, T_], f32, isOutput=True)
    v_final = nc.declare_dram_parameter("v_final", [BL_, T_], f32, isOutput=True)

    if variant == "v2":
        # table stored [j, i] (transposed) so the score buffer is written and
        # reduced fully contiguously in [b, j, i] order
        tbl = np.ascontiguousarray(trans_np.T.reshape(1, T_ * T_), dtype=np.float32)
    else:
        tbl = np.ascontiguousarray(trans_np.reshape(1, T_ * T_), dtype=np.float32)
    tbc_d = nc.inline_tensor(tbl, "tbc")
    iota_d = nc.inline_tensor(
        np.arange(T_ - 1, -1, -1, dtype=np.float32).reshape(1, T_), "iotad"
    )

    with tile.TileContext(nc) as tc:
        with (
            tc.tile_pool(name="const", bufs=1) as cpool,
            tc.tile_pool(name="feat", bufs=8) as fpool,
            tc.tile_pool(name="vst", bufs=4) as vpool,
            tc.tile_pool(name="sc", bufs=1 if variant == "v0" else 2) as scpool,
            tc.tile_pool(name="mx", bufs=2) as mxpool,
            tc.tile_pool(name="bpp", bufs=4) as bppool,
        ):
            tbc = cpool.tile([BL_, T_ * T_], f32, tag="tbc")
            nc.gpsimd.dma_start(tbc[:, :], tbc_d[:, :].partition_broadcast(BL_))
            iotab = cpool.tile([BL_, T_], f32, tag="iotab")
            nc.gpsimd.dma_start(iotab[:, :], iota_d[:, :].partition_broadcast(BL_))

            f0 = fpool.tile([BL_, T_], f32, tag="feat")
            nc.gpsimd.dma_start(f0[:, :], feats[:, 0, :])
            v = vpool.tile([BL_, T_], f32, tag="v")
            nc.vector.tensor_copy(v[:, :], f0[:, :])

            tb3 = tbc[:, :].rearrange("p (i j) -> p i j", i=T_)
            io3 = iotab[:, :].unsqueeze(-1).broadcast_to([BL_, T_, T_])
            # v2: table is [j, i]-major; split the add by j between DVE and
            # Pool (Pool ~2x slower -> give it the smaller range)
            import os as _os
            JSPLIT = int(_os.environ.get("VT_JSPLIT", T_))
            DSPLIT = int(_os.environ.get("VT_DSPLIT", T_ // 2))

            for t in range(1, S_):
                ft = fpool.tile([BL_, T_], f32, tag="feat")
                nc.gpsimd.dma_start(ft[:, :], feats[:, t, :])

                sc = scpool.tile([BL_, T_ * T_], f32, tag="sc")
                sc3 = sc[:, :].rearrange("p (i j) -> p i j", i=T_)
                scT = sc[:, :].rearrange("p (i j) -> p j i", i=T_)
                mxt = mxpool.tile([BL_, T_], f32, tag="mx")

                if variant == "v2":
                    # sc[b, j, i] = v[b, i] + tT[j, i]; contiguous writes
                    scJ = sc[:, :].rearrange("p (j i) -> p j i", j=T_)
                    tbJ = tbc[:, :].rearrange("p (j i) -> p j i", j=T_)
                    nA = JSPLIT * T_
                    v3a = v[:, :].unsqueeze(1).broadcast_to([BL_, JSPLIT, T_])
                    scA = sc[:, 0:nA].rearrange("p (j i) -> p j i", j=JSPLIT)
                    tbA = tbc[:, 0:nA].rearrange("p (j i) -> p j i", j=JSPLIT)
                    nc.vector.tensor_tensor(scA, v3a, tbA, add)
                    if JSPLIT < T_:
                        v3b = v[:, :].unsqueeze(1).broadcast_to(
                            [BL_, T_ - JSPLIT, T_]
                        )
                        scB = sc[:, nA : T_ * T_].rearrange(
                            "p (j i) -> p j i", j=T_ - JSPLIT
                        )
                        tbB = tbc[:, nA : T_ * T_].rearrange(
                            "p (j i) -> p j i", j=T_ - JSPLIT
                        )
                        nc.gpsimd.tensor_tensor(scB, v3b, tbB, add)
                    nc.vector.tensor_reduce(mxt[:, :], scJ, axis=X, op=mx_op)
                elif variant == "v3":
                    # sc[b,i,j] = t[i,j] + v[b,i]: DVE does rows [0, DSPLIT)
                    # in one tensor_tensor; ACT does rows [DSPLIT, T) as
                    # per-row activation-adds (bias = per-partition scalar)
                    nD = DSPLIT * T_
                    v3a = v[:, 0:DSPLIT].unsqueeze(-1).broadcast_to(
                        [BL_, DSPLIT, T_]
                    )
                    scA = sc[:, 0:nD].rearrange("p (i j) -> p i j", i=DSPLIT)
                    tbA = tbc[:, 0:nD].rearrange("p (i j) -> p i j", i=DSPLIT)
                    nc.vector.tensor_tensor(scA, v3a, tbA, add)
                    for i in range(DSPLIT, T_):
                        nc.scalar.add(
                            sc[:, i * T_ : (i + 1) * T_],
                            tbc[:, i * T_ : (i + 1) * T_],
                            v[:, i : i + 1],
                        )
                    nc.vector.tensor_reduce(mxt[:, :], scT, axis=X, op=mx_op)
                else:
                    v3 = v[:, :].unsqueeze(-1).broadcast_to([BL_, T_, T_])
                    nc.vector.tensor_tensor(sc3, v3, tb3, add)
                    nc.vector.tensor_reduce(mxt[:, :], scT, axis=X, op=mx_op)

                vn = vpool.tile([BL_, T_], f32, tag="v")
                nc.vector.tensor_tensor(vn[:, :], mxt[:, :], ft[:, :], add)

                if variant == "v0":
                    # backpointers on device: sc <- (sc==mx)*(T-1-i); bp=max_i
                    mx3 = mxt[:, :].unsqueeze(1).broadcast_to([BL_, T_, T_])
                    nc.vector.tensor_tensor(sc3, sc3, mx3, eq_op)
                    nc.vector.tensor_tensor(sc3, sc3, io3, mul_op)
                    bpt = bppool.tile([BL_, T_], f32, tag="bp")
                    nc.vector.tensor_reduce(bpt[:, :], scT, axis=X, op=mx_op)
                    nc.gpsimd.dma_start(bp[t - 1, :, :], bpt[:, :])
                else:
                    nc.gpsimd.dma_start(vs_out[t - 1, :, :], vn[:, :])

                v = vn

            nc.gpsimd.dma_start(v_final[:, :], v[:, :])
    nc.finalize()
    return nc


def build_viterbi_f16_nc(trans_np, S_=S, T_=T, BL_=BL, kblk=8):
    """fp16 forward-scan kernel: per step, the [T,T] score add and the
    max-tree run in fp16 on DVE (4x perf mode); the state update
    vn = max + feat stays fp32, with per-step recentring (subtract the
    per-row max) so fp16 magnitudes stay ~|8|. The recentred fp32 state
    trajectory streams to DRAM b-major; the host backtraces from it in fp32.

    Numerics validated against reference in numpy sim: ~1e-4 tag mismatch
    rate (rel err ~7e-3, gate is 2e-2).
    """
    import concourse.bacc as bacc
    import concourse.mybir as mybir
    import concourse.tile as tile

    f32 = mybir.dt.float32
    f16 = mybir.dt.float16
    add = mybir.AluOpType.add
    mx_op = mybir.AluOpType.max
    mul_op = mybir.AluOpType.mult
    sub_op = mybir.AluOpType.subtract
    X = mybir.AxisListType.X

    nc = bacc.Bacc("TRN2", target_bir_lowering=False, debug=False)
    feats = nc.declare_dram_parameter("feats", [BL_, S_, T_], f32, isOutput=False)
    vs_out = nc.declare_dram_parameter("vs", [BL_, S_ - 1, T_], f32, isOutput=True)

    # table stored [j, i] (transposed) so score writes and the i-tree are
    # contiguous per j
    tbl16 = np.ascontiguousarray(trans_np.T.reshape(1, T_ * T_)).astype(np.float16)
    tbc_d = nc.inline_tensor(tbl16, "tbc16")

    nblk = (S_ + kblk - 1) // kblk  # feat blocks cover s in [0, S)

    with tile.TileContext(nc) as tc:
        with (
            tc.tile_pool(name="const", bufs=1) as cpool,
            tc.tile_pool(name="feat", bufs=2) as fpool,
            tc.tile_pool(name="vsb", bufs=2) as vspool,
            tc.tile_pool(name="sc", bufs=1) as scpool,
            tc.tile_pool(name="small", bufs=2) as smpool,
        ):
            tbc = cpool.tile([BL_, T_ * T_], f16, tag="tbc")
            nc.gpsimd.dma_start(tbc[:, :], tbc_d[:, :].partition_broadcast(BL_))
            t3 = tbc[:, :].rearrange("p (j i) -> p j i", j=T_)

            s16 = scpool.tile([BL_, T_ * T_], f16, tag="s16")
            s3 = s16[:, :].rearrange("p (j i) -> p j i", j=T_)

            # feat block 0 (s = 0..kblk-1)
            fb = fpool.tile([BL_, kblk * T_], f32, tag="fb")
            nc.gpsimd.dma_start(
                fb[:, :].rearrange("p (k t) -> p k t", k=kblk), feats[:, 0:kblk, :]
            )

            # initial state from f0 (host already folded start_transitions)
            f0 = fb[:, 0:T_]
            shift = smpool.tile([BL_, 1], f32, tag="shift")
            nc.vector.tensor_reduce(shift[:, :], f0, axis=X, op=mx_op)
            v16 = smpool.tile([BL_, T_], f16, tag="v16")
            nc.vector.tensor_scalar(v16[:, :], f0, shift[:, :], None, sub_op)

            vsb = vspool.tile([BL_, kblk * T_], f32, tag="vsb")

            for t in range(1, S_):
                kf, rf = divmod(t, kblk)
                if rf == 0:  # need next feat block (covers s = t..t+kblk-1)
                    fb = fpool.tile([BL_, kblk * T_], f32, tag="fb")
                    hi = min(kblk, S_ - kf * kblk)
                    nc.gpsimd.dma_start(
                        fb[:, 0 : hi * T_].rearrange("p (k t) -> p k t", k=hi),
                        feats[:, kf * kblk : kf * kblk + hi, :],
                    )
                ft = fb[:, rf * T_ : (rf + 1) * T_]

                # s16[b,j,i] = fp16(v16[b,i] + t16[j,i])   (4x DVE mode)
                v3 = v16[:, :].unsqueeze(1).broadcast_to([BL_, T_, T_])
                nc.vector.scalar_tensor_tensor(s3, v3, 1.0, t3, mul_op, add)

                # in-place max tree over i: 128 -> 1
                w = T_ // 2
                while w >= 1:
                    a = s3[:, :, 0:w]
                    b = s3[:, :, w : 2 * w]
                    if w > 1:
                        nc.vector.scalar_tensor_tensor(a, a, 1.0, b, mul_op, mx_op)
                    else:
                        mx16 = smpool.tile([BL_, T_], f16, tag="mx16")
                        m3 = mx16[:, :].rearrange("p (j i) -> p j i", j=T_, i=1)
                        nc.vector.scalar_tensor_tensor(m3, a, 1.0, b, mul_op, mx_op)
                    w //= 2

                # vn32 = fp32(mx16) + ft   -> written into the vs block slot
                r = (t - 1) % kblk
                if r == 0:
                    vsb = vspool.tile([BL_, kblk * T_], f32, tag="vsb")
                vslot = vsb[:, r * T_ : (r + 1) * T_]
                nc.vector.scalar_tensor_tensor(vslot, mx16[:, :], 1.0, ft, mul_op, add)

                # recentre: shift = max_j vn; v16 = fp16(vn - shift)
                shift = smpool.tile([BL_, 1], f32, tag="shift")
                nc.vector.tensor_reduce(shift[:, :], vslot, axis=X, op=mx_op)
                v16 = smpool.tile([BL_, T_], f16, tag="v16")
                nc.vector.tensor_scalar(v16[:, :], vslot, shift[:, :], None, sub_op)

                if r == kblk - 1 or t == S_ - 1:  # flush vs block
                    lo = (t - 1) - r  # first vs row in this block
                    n = r + 1
                    nc.gpsimd.dma_start(
                        vs_out[:, lo : lo + n, :],
                        vsb[:, 0 : n * T_].rearrange("p (k t) -> p k t", k=n),
                    )
    nc.finalize()
    return nc


def build_viterbi_v4_nc(trans_np, S_=S, T_=T, BL_=BL, kblk=8, a_rows=58,
                        apad=64):
    """3-engine fp32 kernel. Exact arithmetic (same single-rounding adds as
    the reference), so tags match bit-exactly.

    Score add s[b,i,j] = t[i,j] + v[b,i] split by i-rows:
      - Act: rows [0, a)   -> s_A buffer, [i, j] layout (row-contig), one
        activation-add per row (bias = v[:, i]). Rows [a, apad) are -1e38
        pad written once so DVE can run a fixed power-of-2 max tree.
      - Pool: rows [a, T)  -> s_P buffer in compact TRANSPOSED [j, k] layout
        (one tensor_tensor add; Pool has no max op, DVE reduces contiguous).
    DVE: in-place contiguous max tree over s_A rows (apad -> 1), one
    contiguous tensor_reduce over s_P, combine, + feat -> vs block slot.
    State trajectory streams b-major; host does the exact fp32 backtrace.
    """
    import concourse.bacc as bacc
    import concourse.mybir as mybir
    import concourse.tile as tile

    f32 = mybir.dt.float32
    add = mybir.AluOpType.add
    mx_op = mybir.AluOpType.max
    mul_op = mybir.AluOpType.mult
    X = mybir.AxisListType.X

    p_rows = T_ - a_rows
    assert a_rows <= apad and (apad & (apad - 1)) == 0

    nc = bacc.Bacc("TRN2", target_bir_lowering=False, debug=False)
    feats = nc.declare_dram_parameter("feats", [BL_, S_, T_], f32, isOutput=False)
    vs_out = nc.declare_dram_parameter("vs", [BL_, S_ - 1, T_], f32, isOutput=True)

    # Act table: rows [0, a) of trans, [i, j] layout
    tblA = np.ascontiguousarray(trans_np[0:a_rows, :].reshape(1, a_rows * T_),
                                dtype=np.float32)
    tblA_d = nc.inline_tensor(tblA, "tblA")
    # Pool table: rows [a, T) transposed-compact: tP[j, k] = trans[a+k, j]
    tblP = np.ascontiguousarray(trans_np[a_rows:, :].T.reshape(1, T_ * p_rows),
                                dtype=np.float32)
    tblP_d = nc.inline_tensor(tblP, "tblP")

    with tile.TileContext(nc) as tc:
        with (
            tc.tile_pool(name="const", bufs=1) as cpool,
            tc.tile_pool(name="feat", bufs=2) as fpool,
            tc.tile_pool(name="vsb", bufs=2) as vspool,
            tc.tile_pool(name="sc", bufs=1) as scpool,
            tc.tile_pool(name="small", bufs=2) as smpool,
        ):
            tbA = cpool.tile([BL_, a_rows * T_], f32, tag="tbA")
            nc.gpsimd.dma_start(tbA[:, :], tblA_d[:, :].partition_broadcast(BL_))
            tbP = cpool.tile([BL_, T_ * p_rows], f32, tag="tbP")
            nc.gpsimd.dma_start(tbP[:, :], tblP_d[:, :].partition_broadcast(BL_))
            tbP3 = tbP[:, :].rearrange("p (j k) -> p j k", j=T_)

            sA = scpool.tile([BL_, apad * T_], f32, tag="sA")
            sA3 = sA[:, :].rearrange("p (i j) -> p i j", i=apad)
            if a_rows < apad:  # one-time -inf pad rows for the fixed tree
                nc.vector.memset(sA[:, a_rows * T_ :], -1.0e38)
            sP = scpool.tile([BL_, T_ * p_rows], f32, tag="sP")
            sP3 = sP[:, :].rearrange("p (j k) -> p j k", j=T_)

            fb = fpool.tile([BL_, kblk * T_], f32, tag="fb")
            nc.gpsimd.dma_start(
                fb[:, :].rearrange("p (k t) -> p k t", k=kblk), feats[:, 0:kblk, :]
            )
            v = fb[:, 0:T_]  # v_0 = feats[:,0] (start folded by host)

            vsb = vspool.tile([BL_, kblk * T_], f32, tag="vsb")

            for t in range(1, S_):
                kf, rf = divmod(t, kblk)
                if rf == 0:
                    fb = fpool.tile([BL_, kblk * T_], f32, tag="fb")
                    hi = min(kblk, S_ - kf * kblk)
                    nc.gpsimd.dma_start(
                        fb[:, 0 : hi * T_].rearrange("p (k t) -> p k t", k=hi),
                        feats[:, kf * kblk : kf * kblk + hi, :],
                    )
                ft = fb[:, rf * T_ : (rf + 1) * T_]

                # --- score adds ---
                for i in range(a_rows):
                    nc.scalar.add(
                        sA[:, i * T_ : (i + 1) * T_],
                        tbA[:, i * T_ : (i + 1) * T_],
                        v[:, i : i + 1],
                    )
                vP = v[:, a_rows:T_].unsqueeze(1).broadcast_to([BL_, T_, p_rows])
                nc.gpsimd.tensor_tensor(sP3, tbP3, vP, add)

                # --- max over i ---
                # in-place contiguous tree over sA rows: apad -> 1
                w = apad // 2
                mxA = smpool.tile([BL_, T_], f32, tag="mxA")
                while w >= 1:
                    i0 = sA3[:, 0:w, :]
                    i1 = sA3[:, w : 2 * w, :]
                    out = i0 if w > 1 else mxA[:, :].rearrange(
                        "p (i j) -> p i j", i=1
                    )
                    nc.vector.scalar_tensor_tensor(out, i0, 1.0, i1, mul_op, mx_op)
                    w //= 2
                mxP = smpool.tile([BL_, T_], f32, tag="mxP")
                nc.vector.tensor_reduce(mxP[:, :], sP3, axis=X, op=mx_op)

                # --- combine + feat -> vs slot (the new v) ---
                r = (t - 1) % kblk
                if r == 0:
                    vsb = vspool.tile([BL_, kblk * T_], f32, tag="vsb")
                vslot = vsb[:, r * T_ : (r + 1) * T_]
                nc.vector.scalar_tensor_tensor(
                    mxA[:, :], mxA[:, :], 1.0, mxP[:, :], mul_op, mx_op
                )
                nc.vector.scalar_tensor_tensor(
                    vslot, mxA[:, :], 1.0, ft, mul_op, add
                )
                v = vslot

                if r == kblk - 1 or t == S_ - 1:
                    lo = (t - 1) - r
                    n = r + 1
                    nc.gpsimd.dma_start(
                        vs_out[:, lo : lo + n, :],
                        vsb[:, 0 : n * T_].rearrange("p (k t) -> p k t", k=n),
                    )
    nc.finalize()
    return nc


def build_viterbi_v5_nc(trans_np, S_=S, T_=T, BL_=BL, kblk=8, a_rows=32,
                        d_rows=16):
    """Pipelined 3-engine fp32 kernel (exact arithmetic).

    Row split of the score add s[b,i,j] = t[i,j] + v[b,i]:
      - Act rows [0, a): per-row activation adds into sA ([i,j] layout).
      - DVE rows [a, a+d) and Pool rows [a+d, T): both write one shared
        compact transposed buffer sDP[b, j, k] (k = i - a), so ONE
        contiguous tensor_reduce covers both regions.
    DVE owns all maxes: in-place tree over sA (a must be a power of two),
    contiguous reduce over sDP in two j-halves, combine + feat per half.
    vn half 0 (j < T/2) is emitted first so Act's next-step rows (i < a <=
    T/2) and DVE's own adds can start while the second half is still being
    reduced — that cross-step overlap is what keeps Act/Pool busy during
    DVE's reduce phase.
    """
    import concourse.bacc as bacc
    import concourse.mybir as mybir
    import concourse.tile as tile

    f32 = mybir.dt.float32
    add = mybir.AluOpType.add
    mx_op = mybir.AluOpType.max
    mul_op = mybir.AluOpType.mult
    X = mybir.AxisListType.X

    p_rows = T_ - a_rows - d_rows
    dp = d_rows + p_rows
    H = T_ // 2
    assert (a_rows & (a_rows - 1)) == 0 and a_rows <= H

    nc = bacc.Bacc("TRN2", target_bir_lowering=False, debug=False)
    feats = nc.declare_dram_parameter("feats", [BL_, S_, T_], f32, isOutput=False)
    vs_out = nc.declare_dram_parameter("vs", [BL_, S_ - 1, T_], f32, isOutput=True)

    tblA = np.ascontiguousarray(trans_np[0:a_rows, :].reshape(1, a_rows * T_),
                                dtype=np.float32)
    tblA_d = nc.inline_tensor(tblA, "tblA")
    # shared compact transposed table: tDP[j, k] = trans[a + k, j]
    tblDP = np.ascontiguousarray(trans_np[a_rows:, :].T.reshape(1, T_ * dp),
                                 dtype=np.float32)
    tblDP_d = nc.inline_tensor(tblDP, "tblDP")

    with tile.TileContext(nc) as tc:
        with (
            tc.tile_pool(name="const", bufs=1) as cpool,
            tc.tile_pool(name="feat", bufs=2) as fpool,
            tc.tile_pool(name="vsb", bufs=2) as vspool,
            tc.tile_pool(name="sc", bufs=1) as scpool,
            tc.tile_pool(name="small", bufs=2) as smpool,
        ):
            tbA = cpool.tile([BL_, a_rows * T_], f32, tag="tbA")
            nc.gpsimd.dma_start(tbA[:, :], tblA_d[:, :].partition_broadcast(BL_))
            tbDP = cpool.tile([BL_, T_ * dp], f32, tag="tbDP")
            nc.gpsimd.dma_start(tbDP[:, :], tblDP_d[:, :].partition_broadcast(BL_))
            tbDP3 = tbDP[:, :].rearrange("p (j k) -> p j k", j=T_)

            sA = scpool.tile([BL_, a_rows * T_], f32, tag="sA")
            sA3 = sA[:, :].rearrange("p (i j) -> p i j", i=a_rows)
            sDP = scpool.tile([BL_, T_ * dp], f32, tag="sDP")
            sDP3 = sDP[:, :].rearrange("p (j k) -> p j k", j=T_)

            fb = fpool.tile([BL_, kblk * T_], f32, tag="fb")
            nc.gpsimd.dma_start(
                fb[:, :].rearrange("p (k t) -> p k t", k=kblk), feats[:, 0:kblk, :]
            )
            v = fb[:, 0:T_]  # v_0 = feats[:,0] (start folded by host)

            vsb = vspool.tile([BL_, kblk * T_], f32, tag="vsb")

            D0, D1 = a_rows, a_rows + d_rows
            for t in range(1, S_):
                kf, rf = divmod(t, kblk)
                if rf == 0:
                    fb = fpool.tile([BL_, kblk * T_], f32, tag="fb")
                    hi = min(kblk, S_ - kf * kblk)
                    nc.gpsimd.dma_start(
                        fb[:, 0 : hi * T_].rearrange("p (k t) -> p k t", k=hi),
                        feats[:, kf * kblk : kf * kblk + hi, :],
                    )
                ft = fb[:, rf * T_ : (rf + 1) * T_]

                # --- score adds (Act needs only vn half 0 of step t-1) ---
                for i in range(a_rows):
                    nc.scalar.add(
                        sA[:, i * T_ : (i + 1) * T_],
                        tbA[:, i * T_ : (i + 1) * T_],
                        v[:, i : i + 1],
                    )
                if d_rows:
                    vD = v[:, D0:D1].unsqueeze(1).broadcast_to([BL_, T_, d_rows])
                    nc.vector.scalar_tensor_tensor(
                        sDP3[:, :, 0:d_rows], tbDP3[:, :, 0:d_rows], 1.0, vD,
                        mul_op, add,
                    )
                vP = v[:, D1:T_].unsqueeze(1).broadcast_to([BL_, T_, p_rows])
                nc.gpsimd.tensor_tensor(
                    sDP3[:, :, d_rows:dp], tbDP3[:, :, d_rows:dp], vP, add
                )

                # --- maxes on DVE ---
                mxA = smpool.tile([BL_, T_], f32, tag="mxA")
                w = a_rows // 2
                while w >= 1:
                    i0 = sA3[:, 0:w, :]
                    i1 = sA3[:, w : 2 * w, :]
                    out = i0 if w > 1 else mxA[:, :].rearrange(
                        "p (i j) -> p i j", i=1
                    )
                    nc.vector.scalar_tensor_tensor(out, i0, 1.0, i1, mul_op, mx_op)
                    w //= 2

                r = (t - 1) % kblk
                if r == 0:
                    vsb = vspool.tile([BL_, kblk * T_], f32, tag="vsb")
                vslot = vsb[:, r * T_ : (r + 1) * T_]
                mxP = smpool.tile([BL_, T_], f32, tag="mxP")
                for h0, h1 in ((0, H), (H, T_)):
                    nc.vector.tensor_reduce(
                        mxP[:, h0:h1], sDP3[:, h0:h1, :], axis=X, op=mx_op
                    )
                    nc.vector.scalar_tensor_tensor(
                        mxA[:, h0:h1], mxA[:, h0:h1], 1.0, mxP[:, h0:h1],
                        mul_op, mx_op,
                    )
                    nc.vector.scalar_tensor_tensor(
                        vslot[:, h0:h1], mxA[:, h0:h1], 1.0, ft[:, h0:h1],
                        mul_op, add,
                    )
                v = vslot

                if r == kblk - 1 or t == S_ - 1:
                    lo = (t - 1) - r
                    n = r + 1
                    nc.gpsimd.dma_start(
                        vs_out[:, lo : lo + n, :],
                        vsb[:, 0 : n * T_].rearrange("p (k t) -> p k t", k=n),
                    )
    nc.finalize()
    return nc


def _install_ntff_hook_shim():
    """The agent image's `antenv` lacks `axon_hooks`, so trn_boot degrades
    silently and bass_utils' trace path crashes on import. Provide the same
    ctypes-based NTFF hook trn_boot would have registered."""
    import sys
    import types

    if "antenv.axon_hooks" in sys.modules:
        return
    try:
        import antenv.axon_hooks  # noqa: F401
        return
    except ImportError:
        pass
    try:
        from trn_agent_boot.trn_boot import _ntff_profile_via_ctypes

        hook = _ntff_profile_via_ctypes("/opt/axon/libaxon_pjrt.so")
    except Exception:
        hook = None
    m = types.ModuleType("antenv.axon_hooks")
    m._hook = hook
    m.get_axon_ntff_profile_hook = lambda: m._hook
    def _set(h):
        m._hook = h
    m.set_axon_ntff_profile_hook = _set
    sys.modules["antenv.axon_hooks"] = m


def _run(nc, in_maps, **kwargs):
    if kwargs.get("trace"):
        _install_ntff_hook_shim()
    from concourse.bass_utils import run_bass_kernel_spmd

    return run_bass_kernel_spmd(nc, in_maps, core_ids=list(range(len(in_maps))), **kwargs)


def _backtrace_from_vs(vs, v0, trans, stop):
    """Exact backtrace from per-step state vectors.

    vs: [B, S-1, T] fp32 (v at t=1..S-1), v0: [B, T] (v at t=0).
    Recomputes argmax_i(v[t-1,:,i] + trans[i, j_t]) along the traced path
    only — identical fp32 arithmetic + first-index ties as the reference.
    """
    B_, Sm1, T_ = vs.shape
    S_ = Sm1 + 1
    last = np.argmax(vs[:, -1, :] + stop[None, :], axis=1).astype(np.int32)
    tags = np.empty((B_, S_), dtype=np.int32)
    tags[:, -1] = last
    cur = last
    transT = np.ascontiguousarray(trans.T)  # [j, i]
    for t in range(S_ - 1, 0, -1):
        vprev = vs[:, t - 2, :] if t >= 2 else v0
        col = vprev + transT[cur]  # [B, T] fp32: v[b,t-1,i] + trans[i, j_t]
        cur = np.argmax(col, axis=1).astype(np.int32)
        tags[:, t - 1] = cur
    return tags


def kernel(feats, transitions, start_transitions, stop_transitions, _trace=False,
           _variant="v2"):
    feats = np.asarray(feats, dtype=np.float32).copy()
    trans = np.ascontiguousarray(np.asarray(transitions, dtype=np.float32))
    start = np.ascontiguousarray(np.asarray(start_transitions, dtype=np.float32))
    stop = np.ascontiguousarray(np.asarray(stop_transitions, dtype=np.float32))
    assert feats.shape == (B, S, T)

    feats[:, 0, :] += start  # fold start_transitions (bit-exact vs reference)

    if _variant == "v5":
        import os as _os
        nc = build_viterbi_v5_nc(
            trans,
            a_rows=int(_os.environ.get("VT_AROWS", "32")),
            d_rows=int(_os.environ.get("VT_DROWS", "16")),
            kblk=int(_os.environ.get("VT_KBLK", "8")),
        )
    elif _variant == "v4":
        import os as _os
        nc = build_viterbi_v4_nc(
            trans,
            a_rows=int(_os.environ.get("VT_AROWS", "58")),
            kblk=int(_os.environ.get("VT_KBLK", "8")),
        )
    elif _variant == "f16":
        nc = build_viterbi_f16_nc(trans)
    else:
        nc = build_viterbi_nc(trans, variant=_variant)
    in_maps = [{"feats": feats[c * BL : (c + 1) * BL]} for c in range(NCORES)]
    res = _run(nc, in_maps, trace=_trace)

    if _variant in ("f16", "v4", "v5"):
        vs = np.concatenate([r["vs"] for r in res.results], axis=0)  # [B, S-1, T]
        v0 = feats[:, 0, :]  # start already folded
        tags = _backtrace_from_vs(vs, v0, trans, stop)
    elif _variant == "v0":
        bp_f = np.concatenate(
            [np.transpose(r["bp"], (1, 0, 2)) for r in res.results], axis=0
        )
        v_fin = np.concatenate([r["v_final"] for r in res.results], axis=0)
        idx = (T - 1) - bp_f.astype(np.int32)
        last = np.argmax(v_fin + stop[None, :], axis=1).astype(np.int32)
        tags = np.empty((B, S), dtype=np.int32)
        tags[:, S - 1] = last
        cur = last
        ar = np.arange(B)
        for t in range(S - 2, -1, -1):
            cur = idx[ar, t, cur]
            tags[:, t] = cur
    else:
        vs = np.concatenate(
            [np.transpose(r["vs"], (1, 0, 2)) for r in res.results], axis=0
        )  # [B, S-1, T]
        v0 = feats[:, 0, :]  # start already folded
        tags = _backtrace_from_vs(vs, v0, trans, stop)

    if _trace:
        return tags, res
    return tags

